# revision 22
# baseline (speedup 1.0000x reference)
"""Capsule-network kernel for 8x TRN2 NeuronCores (data-parallel over batch).

Reference computation (see problem):
  prim = primary_input.reshape(B, 8, 1024)
  prev = zeros(B, 4096)
  for col in 0..3:
    # layer0: inp = [prim_t, x_t, col] (1537) @ W0 -> relu -> flat -> roll(-128)
    # layer1: inp = [x_t, col] (513) @ W1 -> relu -> flat -> roll(+128)
  out = prev @ W_out + b_out

Kernel strategy (per core, batch shard Bc=512):
  - Everything on-chip is FEATURE-MAJOR: tiles are [128 features, Bc batch].
    ROLL=128 == partition count, so rolls are free tile re-indexings.
  - The scalar `col` concat input contributes col*W[last_row] to the
    pre-activation -> folded into per-col biases (computed on host).
  - P = prim @ W0[0:1024] is col-invariant -> computed once (phase 1),
    kept in SBUF, added during the layer0 drain each col.
  - col 0 layer0 has x=0 -> out = relu(P + b0): no matmuls at all.
  - Matmuls run in bf16 (weights, prim, activations; fp32 PSUM accum):
    same 1 cycle/row PE rate as fp32r but half the SBUF/DMA traffic and
    lower PE power (less util-limit throttling). End-to-end rel err vs
    the fp32 reference measured ~4e-3 (budget 2e-2).
"""

import numpy as np

# ---- problem constants (hardcoded; kernel.py must be self-contained) ----
B_FULL = 4096
D_IN = 8192
T = 8            # NUM_TALL
NW = 4           # NUM_WIDE
F = 512          # feature size per capsule row
ROLL = 128
N_CORES = 8
BC = B_FULL // N_CORES   # per-core batch = 512
S = (F * T) // 128       # state feature tiles = 32
KP = (D_IN // T) // 128  # prim k-tiles per capsule row = 8
KX = F // 128            # x k-tiles = 4
FO = F // 128            # output feature tiles per row-layer = 4
N_OUT = 10

_CACHE = {}


def _build_program():
    """Build (and cache) the single-core Bass program. Same program runs
    SPMD on all 8 cores with different batch shards."""
    if "nc" in _CACHE:
        return _CACHE["nc"], _CACHE["names"]

    from contextlib import ExitStack

    import concourse.tile as tile
    from concourse import bacc, mybir

    f32 = mybir.dt.float32
    bf16 = mybir.dt.bfloat16
    AF = mybir.ActivationFunctionType
    ADD = mybir.AluOpType.add

    nc = bacc.Bacc("TRN2", target_bir_lowering=False, debug=False,
                   num_devices=N_CORES)

    prim_d = nc.dram_tensor("prim_t", [D_IN, BC], bf16, kind="ExternalInput").ap()
    w0p_d = nc.dram_tensor("w0p", [KP * 128, F], bf16, kind="ExternalInput").ap()
    w0x_d = nc.dram_tensor("w0x", [F, F], bf16, kind="ExternalInput").ap()
    w1x_d = nc.dram_tensor("w1x", [F, F], bf16, kind="ExternalInput").ap()
    wout_d = nc.dram_tensor("wout_packed", [128, S * N_OUT], bf16,
                            kind="ExternalInput").ap()
    bias0_d = nc.dram_tensor("bias0", [128, NW * FO], f32, kind="ExternalInput").ap()
    bias1_d = nc.dram_tensor("bias1", [128, NW * FO], f32, kind="ExternalInput").ap()
    bout_d = nc.dram_tensor("bout", [N_OUT, 1], f32, kind="ExternalInput").ap()
    out_d = nc.dram_tensor("out", [N_OUT, BC], f32, kind="ExternalOutput").ap()

    with tile.TileContext(nc) as tc, ExitStack() as ctx:
        const = ctx.enter_context(tc.tile_pool(name="const", bufs=1))
        state = ctx.enter_context(tc.tile_pool(name="state", bufs=1))
        cpool = ctx.enter_context(tc.tile_pool(name="cpool", bufs=10))
        prim_pool = ctx.enter_context(tc.tile_pool(name="primp", bufs=10))
        ppool = ctx.enter_context(tc.tile_pool(name="psum", bufs=8, space="PSUM"))

        # ---- constants (tiles only; DMAs are interleaved into phase 1 so
        # the first prim tiles hit SBUF as early as possible) ----
        w0p_sb = [const.tile([128, F], bf16, name=f"w0p{k}", tag=f"w0p{k}")
                  for k in range(KP)]
        w0x_sb = [const.tile([128, F], bf16, name=f"w0x{k}", tag=f"w0x{k}")
                  for k in range(KX)]
        w1x_sb = [const.tile([128, F], bf16, name=f"w1x{k}", tag=f"w1x{k}")
                  for k in range(KX)]
        wout_sb = const.tile([128, S * N_OUT], bf16, name="wout_sb", tag="wout")
        bias0_sb = const.tile([128, NW * FO], f32, name="bias0_sb", tag="bias0")
        bias1_sb = const.tile([128, NW * FO], f32, name="bias1_sb", tag="bias1")
        bout_sb = const.tile([N_OUT, 1], f32, name="bout_sb", tag="bout")

        def load_deferred_consts(t):
            # late-needed constants ride a separate DGE queue (scalar's) so
            # they never sit ahead of prim tiles in the sync queue
            if t == 1:
                # needed from col0-L1 row 0 (~10us later): ride the idle
                # gpsimd SWDGE queue so the hot sync/scalar queues keep all
                # HBM bandwidth for prim during the cold start
                nc.gpsimd.dma_start(bias1_sb[:], bias1_d[:, :])
                nc.gpsimd.dma_start(bout_sb[:], bout_d[:, :])
                for k in range(KX):
                    nc.gpsimd.dma_start(w1x_sb[k][:], w1x_d[k * 128:(k + 1) * 128, :])
            elif t == 3:
                for k in range(KX):
                    nc.scalar.dma_start(w0x_sb[k][:], w0x_d[k * 128:(k + 1) * 128, :])
            elif t == 6:
                nc.scalar.dma_start(wout_sb[:], wout_d[:, :])

        # ---- persistent state ----
        A = [state.tile([128, BC], bf16, name=f"state_a{i}", tag=f"A{i}")
             for i in range(S)]
        P = [state.tile([128, BC], f32, name=f"state_p{i}", tag=f"P{i}")
             for i in range(S)]

        def layer0_row(c, t, Cl):
            for fo in range(FO):
                j = t * FO + fo
                ct = cpool.tile([128, BC], bf16, name=f"c{c}_{j}", tag="C")
                b0ap = bias0_sb[:, c * FO + fo:c * FO + fo + 1]
                if c == 0:
                    # x == 0: out = relu(P + b0)
                    nc.scalar.activation(ct[:], P[j][:], AF.Relu, bias=b0ap)
                else:
                    ps = ppool.tile([128, BC], f32, name=f"ps0_{c}_{j}",
                                    tag="mm")
                    for k in range(KX):
                        x_ap = A[(t * FO + k - 1) % S]
                        nc.tensor.matmul(
                            ps[:],
                            w0x_sb[k][:, fo * 128:(fo + 1) * 128],
                            x_ap[:],
                            start=(k == 0), stop=(k == KX - 1))
                    # ct = (psum + bias0_c) + P  on DVE, then relu on ACT
                    nc.vector.scalar_tensor_tensor(
                        ct[:], ps[:], b0ap, P[j][:], ADD, ADD)
                    nc.scalar.activation(ct[:], ct[:], AF.Relu)
                Cl[j] = ct

        def layer1_row(c, t, Cl, last=False):
            for fo in range(FO):
                j = t * FO + fo
                ps = ppool.tile([128, BC], f32, name=f"ps1_{c}_{j}", tag="mm")
                for k in range(KX):
                    x_ap = Cl[(t * FO + k + 1) % S]
                    nc.tensor.matmul(
                        ps[:],
                        w1x_sb[k][:, fo * 128:(fo + 1) * 128],
                        x_ap[:],
                        start=(k == 0), stop=(k == KX - 1))
                b1ap = bias1_sb[:, c * FO + fo:c * FO + fo + 1]
                # Drain split: in steady-state cols the DVE also carries all
                # the L0 STT drains and periodically saturates (costing one
                # PE slot every ~10us), so ACT takes 3 of 4 L1 drains.  In
                # col 0 (phase 1) and the final row (feeding the out-GEMM
                # tail) ACT is the tight engine, so split evenly there.
                on_act = (fo % 2 == 0) if (c == 0 or last) else (fo != 3)
                if on_act:
                    nc.scalar.activation(A[j][:], ps[:], AF.Relu, bias=b1ap)
                else:
                    # relu(psum + bias) on DVE: (psum add bias) max 0
                    nc.vector.tensor_scalar(A[j][:], ps[:], b1ap, 0.0,
                                            ADD, mybir.AluOpType.max)

        # ---- PE + DMA-ring warmup ----
        # The PE clock p-states ramp 0.65 -> 1.2 -> 2.4 GHz only after ~3us
        # of continuous execution.  While the first prim/w0p DMAs are in
        # flight (~2-3us after the preamble barrier), run dummy matmuls on a
        # zeroed scratch tile so the real row-0 matmuls start at full clock.
        # Tiny leading DMAs absorb the cold-ring wake-up (~1.5us) on both
        # HWDGE queues so the first real tiles transfer sooner.
        warm_sb = const.tile([128, BC], bf16, name="warm_sb", tag="warm")
        ring_sb = const.tile([2, BC], bf16, name="ring_sb", tag="ring")
        nc.sync.dma_start(ring_sb[0:1, :], prim_d[0:1, :])
        nc.scalar.dma_start(ring_sb[1:2, :], w0p_d[0:1, :])
        nc.gpsimd.memset(warm_sb[:], 0.0)
        # 7 dummies x ~460ns (mid clock) bridge the ~3.2us from the preamble
        # barrier to the first prim tile's arrival (~10.8us) with the PE
        # continuously busy, so the 2.4 GHz p-state is reached exactly as
        # real work begins.
        warm_ps = ppool.tile([128, BC], f32, name="ps_warm", tag="mm")
        N_WARM = 7
        for i in range(N_WARM):
            nc.tensor.matmul(warm_ps[:], warm_sb[:, 0:128], warm_sb[:],
                             start=True, stop=True)

        # ---- phase 1 fused with col 0 ----
        # P rows stream in per capsule row (k outer / fo inner so each prim
        # tile is read 4x back-to-back then released). col-0 layer0 is
        # ACT-only (x==0) and col-0 layer1's matmuls have no DMA dependency,
        # so interleaving them gives the PE work while prim streams in.
        Cl0 = [None] * S
        for t in range(T):
            pss = [ppool.tile([128, BC], f32, name=f"ps_p1_{t}_{fo}", tag="mm")
                   for fo in range(FO)]
            if t == 0:
                # row 0: single-tile DMAs for lowest first-matmul latency;
                # w0p rides the scalar queue so it streams concurrently
                # with row-0 prim tiles on the sync queue.  bias0 rides the
                # gpsimd SWDGE (needed by the col0-L0 ACT drain ~10us in).
                nc.gpsimd.dma_start(bias0_sb[:], bias0_d[:, :])
                for k in range(KP):
                    nc.scalar.dma_start(w0p_sb[k][:],
                                        w0p_d[k * 128:(k + 1) * 128, :])
                    pt = prim_pool.tile([128, 2 * BC], bf16, name=f"prim_{k}",
                                        tag="prim")
                    nc.sync.dma_start(pt[:, 0:BC],
                                      prim_d[k * 128:(k + 1) * 128, :])
                    for fo in range(FO):
                        nc.tensor.matmul(
                            pss[fo][:],
                            w0p_sb[k][:, fo * 128:(fo + 1) * 128],
                            pt[:, 0:BC],
                            start=(k == 0), stop=(k == KP - 1))
            else:
                # rows 1-2 (DMA ring still ramping): two k-tiles per DMA for
                # fine-grained arrival; rows 3-7: four k-tiles per DMA (the
                # ring is ~2 rows ahead by then) to halve posting overhead
                # and the per-DMA completion semaphores
                kt = 2 if t <= 2 else 4
                for k2 in range(KP // kt):
                    g = t * KP + kt * k2
                    pt = prim_pool.tile([128, kt * BC], bf16,
                                        name=f"prim_{g}", tag="prim")
                    nc.sync.dma_start(
                        pt[:].rearrange("p (h c) -> p h c", h=kt),
                        prim_d[g * 128:(g + kt) * 128, :].rearrange(
                            "(h p) c -> p h c", p=128))
                    for h in range(kt):
                        k = kt * k2 + h
                        for fo in range(FO):
                            nc.tensor.matmul(
                                pss[fo][:],
                                w0p_sb[k][:, fo * 128:(fo + 1) * 128],
                                pt[:, h * BC:(h + 1) * BC],
                                start=(k == 0), stop=(k == KP - 1))
            load_deferred_consts(t)
            for fo in range(FO):
                j = t * FO + fo
                nc.vector.tensor_copy(P[j][:], pss[fo][:])
                # col-0 layer0 (x==0): C = relu(psum + b0), read directly
                # from PSUM in parallel with the P copy
                ct = cpool.tile([128, BC], bf16, name=f"c0_{j}", tag="C")
                nc.scalar.activation(ct[:], pss[fo][:], AF.Relu,
                                     bias=bias0_sb[:, fo:fo + 1])
                Cl0[j] = ct
            if t >= 1:
                layer1_row(0, t - 1, Cl0)
        layer1_row(0, T - 1, Cl0)

        # ---- cols 1..3 of (layer0, layer1) ----
        # layer1 row t reads C tiles 4t+1..4t+4 (last one produced by layer0
        # row t+1), so emission interleaves: L0(s), L0(s+1), L1(s), L0(s+2),
        # L1(s+1), ..., L1(s+7). The start row rotates by one each col
        # (s = c) so the rows that depend on the previous col's last
        # layer1 writes are emitted last, leaving ~6 rows of pipeline slack
        # across each col boundary.
        for c in range(1, NW):
            Cl = [None] * S
            rows = [(c + i) % T for i in range(T)]
            layer0_row(c, rows[0], Cl)
            for i in range(1, T):
                layer0_row(c, rows[i], Cl)
                layer1_row(c, rows[i - 1], Cl)
            layer1_row(c, rows[T - 1], Cl, last=(c == NW - 1))

        # ---- final: out = prev @ W_out + b_out;  prev[k] = A[(k-1) % S] ----
        psf_full = ppool.tile([128, BC], f32, name="psf", tag="mm")
        psf = psf_full[0:N_OUT, :]
        # emit in col-3's A-write order (rows 3..7,0..2 under the rotation)
        # so the accumulation chain never stalls on the tail of layer1
        ks = [(((3 + i // FO) % T) * FO + i % FO + 1) % S for i in range(S)]
        for i, k in enumerate(ks):
            nc.tensor.matmul(
                psf[:],
                wout_sb[:, k * N_OUT:(k + 1) * N_OUT],
                A[(k - 1) % S][:],
                start=(i == 0), stop=(i == S - 1))
        # drain + store the [10, BC] result in two halves on two engines and
        # two DMA queues so the serial tail is ~halved
        out_sb = cpool.tile([N_OUT, BC], f32, name="out_sb", tag="C")
        h = BC // 2
        nc.scalar.activation(out_sb[:, 0:h], psf[:, 0:h], AF.Identity,
                             bias=bout_sb[:])
        nc.vector.tensor_scalar(out_sb[:, h:], psf[:, h:], bout_sb[:], 0.0,
                                ADD, ADD)
        nc.sync.dma_start(out_d[:, :], out_sb[:])

    nc.compile()

    names = dict(prim="prim_t", w0p="w0p", w0x="w0x", w1x="w1x",
                 wout="wout_packed", bias0="bias0", bias1="bias1",
                 bout="bout", out="out")
    _CACHE["nc"] = nc
    _CACHE["names"] = names
    return nc, names


def _make_in_maps(primary_input, W0, b0, W1, b1, W_out, b_out):
    """Host-side sharding + layout prep (all cheap numpy except the
    feature-major transpose of the batch shards)."""
    primary_input = np.ascontiguousarray(primary_input, dtype=np.float32)
    W0 = np.asarray(W0, dtype=np.float32)
    b0 = np.asarray(b0, dtype=np.float32)
    W1 = np.asarray(W1, dtype=np.float32)
    b1 = np.asarray(b1, dtype=np.float32)
    W_out = np.asarray(W_out, dtype=np.float32)
    b_out = np.asarray(b_out, dtype=np.float32)

    import ml_dtypes
    bf16 = ml_dtypes.bfloat16

    ps = D_IN // T  # 1024
    w0p = np.ascontiguousarray(W0[:ps].astype(bf16))        # [1024, 512]
    w0x = np.ascontiguousarray(W0[ps:ps + F].astype(bf16))  # [512, 512]
    w0_last = W0[ps + F]                             # [512]
    w1x = np.ascontiguousarray(W1[:F].astype(bf16))  # [512, 512]
    w1_last = W1[F]                                  # [512]

    bias0 = np.concatenate(
        [(b0 + c * w0_last).reshape(FO, 128).T for c in range(NW)], axis=1)
    bias1 = np.concatenate(
        [(b1 + c * w1_last).reshape(FO, 128).T for c in range(NW)], axis=1)
    bias0 = np.ascontiguousarray(bias0, dtype=np.float32)   # [128, 16]
    bias1 = np.ascontiguousarray(bias1, dtype=np.float32)   # [128, 16]

    # wout_packed[p, k*10+o] = W_out[128k+p, o]
    wout_packed = np.ascontiguousarray(
        W_out.reshape(S, 128, N_OUT).transpose(1, 0, 2)
        .reshape(128, S * N_OUT).astype(bf16))
    bout = np.ascontiguousarray(b_out.reshape(N_OUT, 1))

    shared = dict(w0p=w0p, w0x=w0x, w1x=w1x, wout_packed=wout_packed,
                  bias0=bias0, bias1=bias1, bout=bout)
    in_maps = []
    for core in range(N_CORES):
        shard = primary_input[core * BC:(core + 1) * BC]          # [512, 8192]
        prim_t = np.ascontiguousarray(shard.T.astype(bf16))       # [8192, 512]
        m = {"prim_t": prim_t}
        m.update(shared)
        in_maps.append(m)
    return in_maps


def _install_ntff_hook():
    """Provide antenv.axon_hooks (absent in this image) backed by ctypes
    calls into libaxon_pjrt.so, so run_bass_kernel_spmd(trace=True) can
    capture NTFF profiles. Mirrors trn_agent_boot.trn_boot."""
    import contextlib
    import ctypes
    import sys
    import types

    if "antenv.axon_hooks" in sys.modules:
        return
    so_path = "/opt/axon/libaxon_pjrt.so"
    lib = ctypes.CDLL(so_path)
    lib.axon_start_nrt_profile.argtypes = [ctypes.POINTER(ctypes.c_int64),
                                           ctypes.c_size_t]
    lib.axon_start_nrt_profile.restype = ctypes.c_int64
    lib.axon_stop_nrt_profile.argtypes = [ctypes.c_char_p]
    lib.axon_stop_nrt_profile.restype = ctypes.c_int64

    @contextlib.contextmanager
    def _hook(output_dir, device_ids):
        import jax
        jax.devices()
        if device_ids:
            ids = (ctypes.c_int64 * len(device_ids))(*device_ids)
            rc = lib.axon_start_nrt_profile(ids, len(device_ids))
        else:
            rc = lib.axon_start_nrt_profile(None, 0)
        if rc != 0:
            raise RuntimeError(f"axon_start_nrt_profile rc={rc}")
        try:
            yield
        finally:
            n = lib.axon_stop_nrt_profile(str(output_dir).encode())
            print(f"profile: {n} file(s) written to {output_dir}",
                  file=sys.stderr)

    mod = types.ModuleType("antenv.axon_hooks")
    mod.get_axon_ntff_profile_hook = lambda: _hook
    mod.set_axon_ntff_profile_hook = lambda h: None
    sys.modules["antenv.axon_hooks"] = mod
    import antenv
    antenv.axon_hooks = mod


def kernel(primary_input, W0, b0, W1, b1, W_out, b_out, _trace=False,
           _trace_cores=None):
    from concourse import bass_utils

    if _trace:
        _install_ntff_hook()

    nc, _ = _build_program()
    in_maps = _make_in_maps(primary_input, W0, b0, W1, b1, W_out, b_out)
    res = bass_utils.run_bass_kernel_spmd(
        nc, in_maps, core_ids=list(range(N_CORES)),
        trace=_trace, trace_cores=_trace_cores)
    out = np.empty((B_FULL, N_OUT), dtype=np.float32)
    for core in range(N_CORES):
        out[core * BC:(core + 1) * BC] = res.results[core]["out"].T
    if _trace:
        kernel._last_results = res
    return out

